# revision 1
# baseline (speedup 1.0000x reference)
"""Trainium2 Bass kernel for AdapterFunnelRelMultiheadAttention (v2).

Sharding: data-parallel over (batch, query-block). 8 cores; core c handles
batch c//4, query rows [(c%4)*256, (c%4)*256+256), all 12 heads. No
collectives; host slices inputs (contraction-major, pre-tiled to the SBUF
partition layout and pre-cast to bf16) and concatenates the 8 outputs.

v2 design:
  - weights/activations pre-cast to bf16 AND pre-tiled host-side into
    [128, nt, w] partition-tile layouts, so every major operand is one
    DMA; small per-partition constants ride one [128, 38] tensor
  - rel-pos band extracted with ONE [128,1024] diagonal-AP DMA per
    (i-tile, head) instead of 8 small ones; cls row mask rides the
    PSUM->SBUF pos copy; cls col mask is folded into host-side ttm*cls
    and a single-column band fix
  - score assembled on the PE (content matmuls + id @ t2 accumulated
    into the same PSUM group); exp reads PSUM directly with the
    token-type 'diff' bias as its per-partition bias operand and
    produces softmax denominators for free via accum_out
  - prob normalized by one per-partition multiply, transposed on the PE;
    PV writes both heads of a pair into one [128,128] PSUM tile
    (stationary column placement) - no reciprocal-broadcast or
    partition-shift DMAs anywhere
  - stage 3 (post-proj/adapter/LN) is split per i-tile and reuses the
    stage-2 PSUM pools; the it=0 chain is interleaved into the second
    half of the attention loop as PE side-work
"""

import math

import numpy as np

B, S, D, H, DH, A = 2, 1024, 768, 12, 64, 64
C = S
N_CORES = 8
IB = (B * S) // N_CORES        # 256 query rows per core
NT_I = IB // 128               # 2 i-tiles
NT_J = C // 128                # 8 j-tiles
NT_D = D // 128                # 6 contraction tiles
NT_H = (H * DH) // 128         # 6 head-pair tiles
TW = 1280                      # padded rel-pos window rows per core
PW = 1152                      # per-i-tile pos window width (512+512+128)
NCONST = 6 * NT_H + NT_I       # packed constants: bq bk bpost rwb rrb rsb rmask
SCALE = 1.0 / (DH ** 0.5)

_CACHE = {}


def _build_graph():
    from contextlib import ExitStack

    import concourse.bass as bass
    import concourse.mybir as mybir
    import concourse.tile as tile
    from concourse import bacc

    f32 = mybir.dt.float32
    bf16 = mybir.dt.bfloat16
    op = mybir.AluOpType
    AF = mybir.ActivationFunctionType
    ds = bass.ds

    nc = bacc.Bacc()

    # ---- per-core DRAM parameters (pre-tiled [128, nt, w] layouts) -----
    qr_p = nc.declare_dram_parameter("q_rows", [128, NT_I, D], f32, isOutput=False)
    qrt_p = nc.declare_dram_parameter("q_rowsT", [128, NT_D, IB], bf16, isOutput=False)
    keyt_p = nc.declare_dram_parameter("keyT", [128, NT_D, C], bf16, isOutput=False)
    valt_p = nc.declare_dram_parameter("valT", [128, NT_D, C], bf16, isOutput=False)
    post_p = nc.declare_dram_parameter("posT", [128, NT_D, TW], bf16, isOutput=False)
    ttm_p = nc.declare_dram_parameter("ttm", [128, NT_I, C], bf16, isOutput=False)
    wqt_p = nc.declare_dram_parameter("wqT", [128, NT_D, H * DH], bf16, isOutput=False)
    wkt_p = nc.declare_dram_parameter("wkT", [128, NT_D, H * DH], bf16, isOutput=False)
    wvt_p = nc.declare_dram_parameter("wvT", [128, NT_D, H * DH], bf16, isOutput=False)
    rk_p = nc.declare_dram_parameter("rk", [128, NT_D, H * DH], bf16, isOutput=False)
    wpt_p = nc.declare_dram_parameter("wpostT", [128, NT_H, D], bf16, isOutput=False)
    wdt_p = nc.declare_dram_parameter("wdownT", [128, NT_D, A], bf16, isOutput=False)
    wut_p = nc.declare_dram_parameter("wupT", [A, D], bf16, isOutput=False)
    segb_p = nc.declare_dram_parameter("seg_b", [128, NT_H, 4], bf16, isOutput=False)
    idm_p = nc.declare_dram_parameter("idm", [128, 128], bf16, isOutput=False)
    const_p = nc.declare_dram_parameter("consts", [128, NCONST], f32, isOutput=False)
    lnx_p = nc.declare_dram_parameter("lnx", [1, 3, D], f32, isOutput=False)
    out_p = nc.declare_dram_parameter("out", [IB, D], f32, isOutput=True)

    out_d = out_p.ap()

    with tile.TileContext(nc) as tc, ExitStack() as ctx:
        per = ctx.enter_context(tc.tile_pool(name="persist", bufs=1))

        # ---- persistent tiles -----------------------------------------
        query_nat = per.tile([128, NT_I, D], f32, tag="qnat", name="qnat")
        qTw = per.tile([128, NT_H, IB], bf16, tag="qTw", name="qTw")
        qTr = per.tile([128, NT_H, IB], bf16, tag="qTr", name="qTr")
        kT = per.tile([128, NT_H, C], bf16, tag="kT", name="kT")
        v_sb = per.tile([128, NT_J, H * DH], bf16, tag="vsb", name="vsb")
        rhT = per.tile([128, NT_H, TW], bf16, tag="rhT", name="rhT")
        ttm_bf = per.tile([128, NT_I, C], bf16, tag="ttm", name="ttm")
        segB = per.tile([128, NT_H, 4], bf16, tag="segB", name="segB")
        id_bf = per.tile([128, 128], bf16, tag="id_bf", name="id_bf")
        avT = per.tile([128, NT_H, IB], bf16, tag="avT", name="avT")
        aoT = per.tile([128, NT_H, IB], bf16, tag="aoT", name="aoT")
        a2T = per.tile([128, NT_H, IB], bf16, tag="a2T", name="a2T")
        wpT = per.tile([128, NT_H, D], bf16, tag="wpT", name="wpT")
        wdT = per.tile([128, NT_D, A], bf16, tag="wdT", name="wdT")
        wuT = per.tile([64, D], bf16, tag="wuT", name="wuT")
        consts = per.tile([128, NCONST], f32, tag="consts", name="consts")
        ttsc = per.tile([128, NT_I * NT_H * 4], f32, tag="ttsc", name="ttsc")
        sd_all = per.tile([128, NT_I * NT_H * 2], f32, tag="sdall", name="sdall")
        dd_all = per.tile([128, NT_I * NT_H * 2], f32, tag="ddall", name="ddall")
        qb_w = per.tile([128, NT_H], f32, tag="qb_w", name="qb_w")
        qb_r = per.tile([128, NT_H], f32, tag="qb_r", name="qb_r")
        qb_s = per.tile([128, NT_H], f32, tag="qb_s", name="qb_s")
        lnx_row = per.tile([1, 3, D], f32, tag="lnxr", name="lnxr")
        lnx_bc = per.tile([128, 3, D], f32, tag="lnx_bc", name="lnx_bc")
        eps_t = per.tile([128, 1], f32, tag="eps", name="eps")

        # packed constant views
        bq_t = consts[:, 0 * NT_H:1 * NT_H]
        bk_t = consts[:, 1 * NT_H:2 * NT_H]
        bpost_t = consts[:, 2 * NT_H:3 * NT_H]
        rwb_t = consts[:, 3 * NT_H:4 * NT_H]
        rrb_t = consts[:, 4 * NT_H:5 * NT_H]
        rsb_t = consts[:, 5 * NT_H:6 * NT_H]
        rmask_sb = consts[:, 6 * NT_H:6 * NT_H + NT_I]
        bv_bc = lnx_bc[:, 0, :]
        lnw_bc = lnx_bc[:, 1, :]
        lnb_bc = lnx_bc[:, 2, :]

        # =============== stage 0/1: loads + projections =================
        with (
            tc.tile_pool(name="ps1", bufs=2, space="PSUM") as ps1,
            tc.tile_pool(name="w1", bufs=1) as w1,
        ):
            queryT = w1.tile([128, NT_D, IB], bf16, tag="queryT", name="queryT")
            wqT = w1.tile([128, NT_D, H * DH], bf16, tag="wqT", name="wqT")
            keyT = w1.tile([128, NT_D, C], bf16, tag="keyT", name="keyT")
            wkT = w1.tile([128, NT_D, H * DH], bf16, tag="wkT", name="wkT")
            valT = w1.tile([128, NT_D, C], bf16, tag="valT", name="valT")
            wvT = w1.tile([128, NT_D, H * DH], bf16, tag="wvT", name="wvT")
            posT = w1.tile([128, NT_D, TW], bf16, tag="posT", name="posT")
            rkb = w1.tile([128, NT_D, H * DH], bf16, tag="rkb", name="rkb")
            qTs = w1.tile([128, NT_H, IB], bf16, tag="qTs", name="qTs")

            # all loads on one ring, in strict need-order, one DMA each
            nc.sync.dma_start(out=queryT, in_=qrt_p.ap())
            nc.sync.dma_start(out=wqT, in_=wqt_p.ap())
            nc.sync.dma_start(out=consts, in_=const_p.ap())
            nc.sync.dma_start(out=segB, in_=segb_p.ap())
            nc.sync.dma_start(out=keyT, in_=keyt_p.ap())
            nc.sync.dma_start(out=wkT, in_=wkt_p.ap())
            nc.sync.dma_start(out=posT, in_=post_p.ap())
            nc.sync.dma_start(out=rkb, in_=rk_p.ap())
            nc.sync.dma_start(out=valT, in_=valt_p.ap())
            nc.sync.dma_start(out=wvT, in_=wvt_p.ap())
            nc.sync.dma_start(out=ttm_bf, in_=ttm_p.ap())
            nc.sync.dma_start(out=id_bf, in_=idm_p.ap())
            nc.sync.dma_start(out=wpT, in_=wpt_p.ap())
            nc.sync.dma_start(out=wdT, in_=wdt_p.ap())
            nc.sync.dma_start(out=wuT, in_=wut_p.ap())
            nc.sync.dma_start(out=lnx_row, in_=lnx_p.ap())
            nc.gpsimd.partition_broadcast(lnx_bc, lnx_row)
            nc.sync.dma_start(out=query_nat, in_=qr_p.ap())

            nc.vector.memset(eps_t, 1e-9)
            # preload activation tables off the critical path
            warm = w1.tile([1, 1], f32, tag="warm", name="warm", bufs=1)
            nc.scalar.activation(out=warm, in_=eps_t[0:1, 0:1], func=AF.Gelu)
            nc.scalar.activation(out=warm, in_=eps_t[0:1, 0:1], func=AF.Sqrt)
            nc.scalar.activation(out=warm, in_=eps_t[0:1, 0:1], func=AF.Exp)
            for qb, rb in ((qb_w, rwb_t), (qb_r, rrb_t), (qb_s, rsb_t)):
                nc.vector.tensor_tensor(qb, bq_t, rb, op.add)
                nc.vector.tensor_scalar(qb, qb, SCALE, None, op.mult)

            # q^T variants: (hd, i). Interleave ht pairs so consecutive
            # accumulating matmuls hit different PSUM banks (same-bank
            # accumulation serializes on the RMW and runs at half rate)
            for hp2 in range(NT_H // 2):
                qps = [ps1.tile([128, IB], f32, tag="projq", name=f"projq{e}", bufs=3)
                       for e in range(2)]
                for dt in range(NT_D):
                    for e in range(2):
                        nc.tensor.matmul(qps[e], wqT[:, dt, ds((2 * hp2 + e) * 128, 128)],
                                         queryT[:, dt, :],
                                         start=(dt == 0), stop=(dt == NT_D - 1))
                for e in range(2):
                    ht = 2 * hp2 + e
                    nc.vector.tensor_scalar(qTw[:, ht, :], qps[e], SCALE, qb_w[:, ht:ht + 1], op.mult, op.add)
                    nc.vector.tensor_scalar(qTr[:, ht, :], qps[e], SCALE, qb_r[:, ht:ht + 1], op.mult, op.add)
                    nc.vector.tensor_scalar(qTs[:, ht, :], qps[e], SCALE, qb_s[:, ht:ht + 1], op.mult, op.add)

            # token-type bias scalars for all (it, head-pair):
            # cols [4g+2e+0, 4g+2e+1] = (diff, same) for unit (g, e)
            tt_ps = ps1.tile([128, NT_I * NT_H * 4], f32, tag="tt2", name="tt2", bufs=1)
            for it in range(NT_I):
                for ht in range(NT_H):
                    g = it * NT_H + ht
                    nc.tensor.matmul(tt_ps[:, ds(g * 4, 4)],
                                     qTs[:, ht, ds(it * 128, 128)],
                                     segB[:, ht, :], start=True, stop=True)
            nc.vector.tensor_copy(out=ttsc, in_=tt_ps)
            tt4 = ttsc.rearrange("p (g e t) -> p g e t", e=2, t=2)
            nc.vector.tensor_tensor(sd_all.rearrange("p (g e) -> p g e", e=2),
                                    tt4[:, :, :, 1], tt4[:, :, :, 0], op.subtract)
            # dd = diff * rowmask (per-unit exp bias)
            for it in range(NT_I):
                usl = ds(it * NT_H * 2, NT_H * 2)
                nc.vector.tensor_scalar(
                    dd_all[:, usl].rearrange("p (g e) -> p g e", e=2),
                    tt4[:, it * NT_H:(it + 1) * NT_H, :, 0],
                    rmask_sb[:, it:it + 1], None, op.mult)

            # k^T: (hd, j)
            for ht in range(NT_H):
                k_ps = ps1.tile([128, C], f32, tag="proj", name="projk")
                for dt in range(NT_D):
                    for nh in range(2):
                        nc.tensor.matmul(k_ps[:, ds(nh * 512, 512)],
                                         wkT[:, dt, ds(ht * 128, 128)],
                                         keyT[:, dt, ds(nh * 512, 512)],
                                         start=(dt == 0), stop=(dt == NT_D - 1))
                nc.vector.tensor_scalar(kT[:, ht, :], k_ps, bk_t[:, ht:ht + 1], None, op.add)

            # r_head^T: (hd, t)
            for ht in range(NT_H):
                for th in range(2):
                    r_full = ps1.tile([128, C], f32, tag="proj", name="rproj")
                    r_ps = r_full[:, 0:TW // 2]
                    for dt in range(NT_D):
                        for (o, w) in ((0, 512), (512, 128)):
                            nc.tensor.matmul(r_ps[:, ds(o, w)],
                                             rkb[:, dt, ds(ht * 128, 128)],
                                             posT[:, dt, ds(th * 640 + o, w)],
                                             start=(dt == 0), stop=(dt == NT_D - 1))
                    nc.scalar.copy(out=rhT[:, ht, ds(th * 640, 640)], in_=r_ps)

            # v natural: (j, hd) with fused bias; head-pair ht occupies
            # cols [128ht, 128ht+128) = [head 2ht | head 2ht+1]
            for jt in range(NT_J):
                v_ps = ps1.tile([128, H * DH], f32, tag="proj", name="projv")
                for dt in range(NT_D):
                    for (o, w) in ((0, 512), (512, 256)):
                        nc.tensor.matmul(v_ps[:, ds(o, w)],
                                         valT[:, dt, ds(jt * 128, 128)],
                                         wvT[:, dt, ds(o, w)],
                                         start=(dt == 0), stop=(dt == NT_D - 1))
                nc.vector.tensor_tensor(v_sb[:, jt, :], v_ps, bv_bc, op.add)

        # =============== stage 2 + interleaved stage 3 ==================
        # unit u = (it, ht, e); software pipeline A(u) / B(u-1) / C(u-2).
        # stage-3 per-i-tile chains reuse the stage-2 PSUM pools; the it=0
        # chain is issued as side-work during the it=1 attention units.
        units = [(it, ht, e) for it in range(NT_I) for ht in range(NT_H) for e in range(2)]

        with (
            tc.tile_pool(name="ps2c", bufs=3, space="PSUM") as ps2c,
            tc.tile_pool(name="ps2p", bufs=3, space="PSUM") as ps2p,
            tc.tile_pool(name="ps2t", bufs=1, space="PSUM") as ps2t,
            tc.tile_pool(name="ps2a", bufs=1, space="PSUM") as ps2a,
            tc.tile_pool(name="w2", bufs=6) as w2,
            tc.tile_pool(name="w3", bufs=2) as w3,
        ):
            state = {}

            def stage_a(u):
                it, ht, e = units[u]
                isl = ds(it * 128, 128)
                hp = e * DH
                o0 = 128 * (1 - it)
                # pos scores over the 1152-wide window, one 1-bank psum
                # chunk at a time so every stage-2 pool double-buffers
                pos_sb = w2.tile([128, PW], bf16, tag="possb", name=f"possb{u}")
                for ci, (o, w) in enumerate(((0, 512), (512, 512), (1024, PW - 1024))):
                    p_ps = ps2p.tile([128, 512], f32, tag="pps", name=f"pps{u}_{ci}")
                    nc.tensor.matmul(p_ps[:, 0:w],
                                     qTr[hp:hp + DH, ht, isl],
                                     rhT[hp:hp + DH, ht, ds(o0 + o, w)],
                                     start=True, stop=True)
                    # psum -> sbuf (bf16) with cls row-mask fused
                    if ci == 0:
                        nc.vector.tensor_scalar(pos_sb[:, ds(o, w)], p_ps[:, 0:w],
                                                rmask_sb[:, it:it + 1], None, op.mult)
                    else:
                        nc.scalar.activation(out=pos_sb[:, ds(o, w)], in_=p_ps[:, 0:w],
                                             func=AF.Copy, scale=rmask_sb[:, it:it + 1])
                # diagonal band extraction: band[p, j] = pos_sb[p, 127 - p + j]
                band = w2.tile([128, C], bf16, tag="band", name=f"band{u}")
                diag = bass.AP(tensor=pos_sb.tensor, offset=pos_sb.offset + 127,
                               ap=[[PW - 1, 128], [1, C]])
                nc.sync.dma_start(out=band, in_=diag)
                # cls col-0: band[:,0] = -dd so exp's +dd bias cancels there
                nc.vector.tensor_scalar(band[:, 0:1], dd_all[:, u:u + 1], -1.0,
                                        None, op.mult)
                state[u] = band

            def stage_b(u):
                it, ht, e = units[u]
                isl = ds(it * 128, 128)
                hp = e * DH
                band = state.pop(u)
                if e == 0:
                    state[("av", it, ht)] = ps2a.tile(
                        [128, 128], f32, tag="av", name=f"av{it}_{ht}")
                # t2 = ttm*cls*(same-diff) + band
                t2 = w2.tile([128, C], bf16, tag="t2", name=f"t2{u}")
                nc.vector.scalar_tensor_tensor(t2, ttm_bf[:, it, :], sd_all[:, u:u + 1],
                                               band, op.mult, op.add)
                # score assembled on the PE: content + id@t2, then exp straight
                # from PSUM with the token-type diff bias and free denominators
                prob = w2.tile([128, C], bf16, tag="prob", name=f"prob{u}")
                dens = w2.tile([128, 2], f32, tag="dens", name=f"dens{u}")
                cps = [ps2c.tile([128, 512], f32, tag="cps", name=f"cps{u}_{hf}")
                       for hf in range(2)]
                # content then id@t2, bank-alternating to avoid RMW stalls
                for hf in range(2):
                    nc.tensor.matmul(cps[hf], qTw[hp:hp + DH, ht, isl],
                                     kT[hp:hp + DH, ht, ds(hf * 512, 512)],
                                     start=True, stop=False)
                for hf in range(2):
                    nc.tensor.matmul(cps[hf], id_bf, t2[:, ds(hf * 512, 512)],
                                     start=False, stop=True)
                for hf in range(2):
                    nc.scalar.activation(out=prob[:, ds(hf * 512, 512)], in_=cps[hf],
                                         func=AF.Exp, bias=dd_all[:, u:u + 1],
                                         accum_out=dens[:, hf:hf + 1])
                rd = w2.tile([128, 1], f32, tag="rd", name=f"rd{u}")
                nc.vector.tensor_tensor(rd, dens[:, 0:1], dens[:, 1:2], op.add)
                nc.vector.reciprocal(rd, rd)
                prob_n = w2.tile([128, C], bf16, tag="probn", name=f"probn{u}")
                nc.vector.tensor_scalar(prob_n, prob, rd[:, 0:1], None, op.mult)
                state[u] = prob_n

            def stage_c(u):
                it, ht, e = units[u]
                prob_n = state.pop(u)
                av = state[("av", it, ht)]
                hp = e * DH
                pT_ps = ps2t.tile([128, NT_J, 128], bf16, tag="pT", name=f"pT{u}")
                for jc in range(NT_J):
                    nc.tensor.transpose(pT_ps[:, jc, :], prob_n[:, ds(jc * 128, 128)],
                                        id_bf)
                probT = w2.tile([128, NT_J, 128], bf16, tag="probT", name=f"probT{u}")
                nc.vector.tensor_copy(out=probT[:, 0:4, :], in_=pT_ps[:, 0:4, :])
                nc.scalar.copy(out=probT[:, 4:8, :], in_=pT_ps[:, 4:8, :])
                # PV accumulation; head e lands on psum partitions [64e, 64e+64)
                for jt in range(NT_J):
                    nc.tensor.matmul(av[hp:hp + DH, :],
                                     v_sb[:, jt, ds(ht * 128 + hp, DH)],
                                     probT[:, jt, :],
                                     start=(jt == 0), stop=(jt == NT_J - 1))
                if e == 1:
                    avp = state.pop(("av", it, ht))
                    nc.vector.tensor_copy(out=avT[:, ht, ds(it * 128, 128)], in_=avp)

            # ---- stage-3 side-work chain for one i-tile ----------------
            def stage3_work(it):
                isl = ds(it * 128, 128)
                # post-projection per output tile. PSUM comes from the "pps"
                # tag: its buffer frees are vector ops issued earlier in the
                # queue, so side-work never waits on future PE work.
                for et in range(NT_H):
                    po_ps = ps2p.tile([128, 512], f32, tag="pps", name=f"po{it}_{et}")
                    for kc in range(NT_H):
                        nc.tensor.matmul(po_ps[:, 0:128], wpT[:, kc, ds(et * 128, 128)],
                                         avT[:, kc, isl],
                                         start=(kc == 0), stop=(kc == NT_H - 1))
                    nc.vector.tensor_scalar(aoT[:, et, isl], po_ps[:, 0:128],
                                            bpost_t[:, et:et + 1], None, op.add)
                    yield
                # adapter down -> gelu(tanh) -> up -> residual
                z_full = ps2p.tile([128, 512], f32, tag="pps", name=f"z{it}")
                z_ps = z_full[:, 0:128]
                for kc in range(NT_D):
                    nc.tensor.matmul(z_ps[0:DH, :], wdT[:, kc, 0:A], aoT[:, kc, isl],
                                     start=(kc == 0), stop=(kc == NT_D - 1))
                gT = w3.tile([DH, 128], bf16, tag="gT", name=f"gT{it}")
                nc.scalar.activation(out=gT, in_=z_ps[0:DH, :], func=AF.Gelu)
                yield
                for et in range(NT_H):
                    u_ps = ps2p.tile([128, 512], f32, tag="pps", name=f"up{it}_{et}")
                    nc.tensor.matmul(u_ps[:, 0:128], wuT[0:A, ds(et * 128, 128)], gT,
                                     start=True, stop=True)
                    nc.vector.tensor_tensor(a2T[:, et, isl], u_ps[:, 0:128],
                                            aoT[:, et, isl], op.add)
                    yield
                # transpose back to natural layout on the PE
                n_ps = ps2t.tile([128, NT_J, 128], bf16, tag="pT", name=f"trn{it}")
                for et in range(NT_H):
                    nc.tensor.transpose(n_ps[:, et, :], a2T[:, et, isl], id_bf)
                a2n = w3.tile([128, D], bf16, tag="a2n", name=f"a2n{it}")
                nc.vector.tensor_copy(out=a2n, in_=n_ps[:, 0:NT_H, :])
                yield
                x = w3.tile([128, D], f32, tag="x", name=f"x{it}")
                nc.vector.tensor_tensor(x, query_nat[:, it, :], a2n, op.add)
                stats = w3.tile([128, 3, 6], f32, tag="stats", name=f"stats{it}")
                for c3 in range(3):
                    nc.vector.bn_stats(stats[:, c3, :], x[:, ds(c3 * 256, 256)])
                mv = w3.tile([128, 2], f32, tag="mv", name=f"mv{it}")
                nc.vector.bn_aggr(mv, stats)
                sstd = w3.tile([128, 1], f32, tag="sstd", name=f"sstd{it}")
                nc.scalar.activation(out=sstd, in_=mv[:, 1:2], func=AF.Sqrt,
                                     bias=eps_t[:, 0:1], scale=1.0)
                rstd = w3.tile([128, 1], f32, tag="rstd", name=f"rstd{it}")
                nc.vector.reciprocal(rstd, sstd)
                xa = w3.tile([128, D], f32, tag="xa", name=f"xa{it}")
                nc.vector.scalar_tensor_tensor(xa, x, mv[:, 0:1], lnw_bc,
                                               op.subtract, op.mult)
                ot = w3.tile([128, D], f32, tag="ot", name=f"ot{it}")
                nc.vector.scalar_tensor_tensor(ot, xa, rstd, lnb_bc, op.mult, op.add)
                nc.sync.dma_start(out=out_d[ds(it * 128, 128), :], in_=ot)
                yield

            side = None
            nu = len(units)
            for u in range(nu):
                stage_a(u)
                if u >= 1:
                    stage_b(u - 1)
                if u >= 3:
                    stage_c(u - 3)
                    if units[u - 3] == (0, NT_H - 1, 1):
                        side = stage3_work(0)   # it=0 attention fully done
                    if side is not None:
                        next(side, None)
            stage_b(nu - 1)
            for u in range(nu - 3, nu):
                stage_c(u)
            if side is not None:
                for _ in side:
                    pass
            for _ in stage3_work(1):
                pass

    nc.compile()
    return nc


def _make_seg_b(seg):
    """Block-diagonal seg operand: one (128, 4) tile per head pair so the
    token-type bias matmul contracts over the full 128 partitions."""
    sb = np.zeros((128, NT_H, 4), np.float32)
    for ht in range(NT_H):
        h0, h1 = 2 * ht, 2 * ht + 1
        sb[0:DH, ht, 0] = seg[0, h0 * DH:(h0 + 1) * DH]
        sb[0:DH, ht, 1] = seg[1, h0 * DH:(h0 + 1) * DH]
        sb[DH:128, ht, 2] = seg[0, h1 * DH:(h1 + 1) * DH]
        sb[DH:128, ht, 3] = seg[1, h1 * DH:(h1 + 1) * DH]
    return sb


def _reference_numpy(inputs):
    """Exact numpy fallback (never hit for the spec'd generator; guards the
    amask==1 / factorizable-cls assumptions)."""
    f = np.float64
    q = np.asarray(inputs["query"], f)
    k = np.asarray(inputs["key"], f)
    v = np.asarray(inputs["value"], f)
    pe = np.asarray(inputs["pos_embed"], f)
    ttm = np.asarray(inputs["token_type_mat"]).astype(bool)
    am = np.asarray(inputs["attention_mask"], f)
    cls = np.asarray(inputs["cls_mask"], f)
    wq, bq = np.asarray(inputs["wq"], f), np.asarray(inputs["bq"], f)
    wk, bk = np.asarray(inputs["wk"], f), np.asarray(inputs["bk"], f)
    wv, bv = np.asarray(inputs["wv"], f), np.asarray(inputs["bv"], f)
    rwb, rrb = np.asarray(inputs["r_w_bias"], f), np.asarray(inputs["r_r_bias"], f)
    rsb = np.asarray(inputs["r_s_bias"], f)
    rk = np.asarray(inputs["r_kernel"], f)
    seg = np.asarray(inputs["seg_embed"], f)
    wpo, bpo = np.asarray(inputs["w_post"], f), np.asarray(inputs["b_post"], f)
    wd, wu = np.asarray(inputs["w_down"], f), np.asarray(inputs["w_up"], f)
    lw, lb = np.asarray(inputs["ln_w"], f), np.asarray(inputs["ln_b"], f)
    b, s, d = q.shape
    c = k.shape[1]
    h, dh = rwb.shape
    scale = 1.0 / dh ** 0.5
    qp = (q @ wq.T + bq).reshape(b, s, h, dh) * scale
    kp = (k @ wk.T + bk).reshape(b, c, h, dh)
    vp = (v @ wv.T + bv).reshape(b, c, h, dh)
    content = np.einsum('bind,bjnd->bnij', qp + rwb * scale, kp)
    r_head = np.einsum('td,dnh->tnh', pe, rk)
    pos = np.einsum('binh,tnh->bnit', qp + rrb * scale, r_head)
    t = pos.shape[-1]
    pos = pos.reshape(b, h, t, s)[:, :, 1:, :].reshape(b, h, s, t - 1)[..., :c]
    pos = pos * cls
    ttb = np.einsum('bind,snd->bnis', qp + rsb * scale, seg)
    tta = np.where(ttm[:, None], ttb[..., 1:2], ttb[..., 0:1]) * cls
    sc = content + pos + tta - 1e6 * (1.0 - am[:, None, None])
    sc = sc - sc.max(-1, keepdims=True)
    p = np.exp(sc)
    p /= p.sum(-1, keepdims=True)
    av = np.einsum('bnij,bjnd->bind', p, vp).reshape(b, s, h * dh)
    ao = av @ wpo.T + bpo
    z = ao @ wd.T
    erf = np.vectorize(math.erf)
    g = z * 0.5 * (1.0 + erf(z / np.sqrt(2.0)))
    ao = g @ wu.T + ao
    x = q + ao
    mu = x.mean(-1, keepdims=True)
    var = x.var(-1, keepdims=True)
    return ((x - mu) / np.sqrt(var + 1e-9) * lw + lb).astype(np.float32)


def _pack_consts(inputs, rowmask, i0):
    """[128, NCONST] f32: bq bk bpost rwb rrb rsb (6 x NT_H cols) + rowmask."""
    f = np.float32
    cst = np.zeros((128, NCONST), f)
    for ci, key in enumerate(("bq", "bk", "b_post")):
        cst[:, ci * NT_H:(ci + 1) * NT_H] = (
            np.asarray(inputs[key], f).reshape(NT_H, 128).T)
    for ci, key in enumerate(("r_w_bias", "r_r_bias", "r_s_bias")):
        cst[:, (3 + ci) * NT_H:(4 + ci) * NT_H] = (
            np.asarray(inputs[key], f).reshape(NT_H, 128).T)
    cst[:, 6 * NT_H:] = rowmask[i0:i0 + IB].reshape(NT_I, 128).T
    return cst


def _tile_p(x, w):
    """[128*nt, w] -> [128, nt, w] partition-tile layout."""
    nt = x.shape[0] // 128
    return np.ascontiguousarray(x.reshape(nt, 128, w).transpose(1, 0, 2))


def _shard_inputs(inputs):
    """Slice full inputs into 8 per-core input maps (contraction-major,
    pre-tiled and pre-cast to bf16 host-side)."""
    import ml_dtypes
    f = np.float32
    bf = ml_dtypes.bfloat16

    def tb(x, w):  # bf16 + partition-tiled
        return _tile_p(np.asarray(x, f).astype(bf), w)

    q = np.asarray(inputs["query"], dtype=f)
    k = np.asarray(inputs["key"], dtype=f)
    v = np.asarray(inputs["value"], dtype=f)
    pe = np.asarray(inputs["pos_embed"], dtype=f)
    ttm = np.asarray(inputs["token_type_mat"]).astype(f)
    cls = np.asarray(inputs["cls_mask"], dtype=f)

    # factor cls = outer(rowmask, colmask); verified by _check_fastpath
    jref = int(np.argmax(cls[S // 2]))
    rowmask = cls[:, jref].copy()
    lnx = np.stack([np.asarray(inputs["bv"], f), np.asarray(inputs["ln_w"], f),
                    np.asarray(inputs["ln_b"], f)])[None]
    shared = {
        "wqT": tb(np.asarray(inputs["wq"], f).T, H * DH),
        "wkT": tb(np.asarray(inputs["wk"], f).T, H * DH),
        "wvT": tb(np.asarray(inputs["wv"], f).T, H * DH),
        "rk": tb(np.asarray(inputs["r_kernel"], f).reshape(D, H * DH), H * DH),
        "wpostT": tb(np.asarray(inputs["w_post"], f).T, D),
        "wdownT": tb(np.asarray(inputs["w_down"], f).T, A),
        "wupT": np.asarray(inputs["w_up"], f).T.astype(bf),
        "seg_b": _make_seg_b(np.asarray(inputs["seg_embed"], f).reshape(2, H * DH)).astype(bf),
        "idm": np.eye(128, dtype=f).astype(bf),
        "lnx": lnx,
    }
    in_maps = []
    for cidx in range(N_CORES):
        b, i0 = cidx // (N_CORES // B), (cidx % (N_CORES // B)) * IB
        win = pe[769 - i0: 2048 - i0]
        if win.shape[0] < TW:
            win = np.concatenate([win, np.zeros((TW - win.shape[0], D), f)], axis=0)
        m = dict(shared)
        m["q_rows"] = _tile_p(q[b, i0:i0 + IB], D)
        m["q_rowsT"] = tb(q[b, i0:i0 + IB].T, IB)
        m["keyT"] = tb(k[b].T, C)
        m["valT"] = tb(v[b].T, C)
        m["posT"] = tb(win.T, TW)
        m["ttm"] = _tile_p((ttm[b, i0:i0 + IB] * cls[i0:i0 + IB]).astype(bf), C)
        m["consts"] = _pack_consts(inputs, rowmask, i0)
        in_maps.append(m)
    return in_maps


def _check_fastpath(inputs):
    cls = np.asarray(inputs["cls_mask"], dtype=np.float32)
    am = np.asarray(inputs["attention_mask"], dtype=np.float32)
    if not np.all(am == 1.0):
        return False
    iref = int(np.argmax(cls[:, S // 2]))
    jref = int(np.argmax(cls[S // 2]))
    rowmask = cls[:, jref]
    colmask = cls[iref, :]
    if not np.array_equal(cls, np.outer(rowmask, colmask)):
        return False
    # graph hardcodes: col mask zero exactly at j=0, one elsewhere
    if not (colmask[0] == 0.0 and np.all(colmask[1:] == 1.0)):
        return False
    return True


def _run(inputs, trace=False):
    from concourse.bass_utils import run_bass_kernel_spmd

    if "nc" not in _CACHE:
        _CACHE["nc"] = _build_graph()
    nc = _CACHE["nc"]
    in_maps = _shard_inputs(inputs)
    res = run_bass_kernel_spmd(nc, in_maps, core_ids=list(range(N_CORES)), trace=trace)
    out = np.empty((B, S, D), np.float32)
    for c in range(N_CORES):
        b, i0 = c // (N_CORES // B), (c % (N_CORES // B)) * IB
        out[b, i0:i0 + IB] = res.results[c]["out"]
    return out, res


def kernel(**inputs):
    if not _check_fastpath(inputs):
        return _reference_numpy(inputs)
    out, _ = _run(inputs, trace=False)
    return out



# revision 5
# speedup vs baseline: 1.2579x; 1.2579x over previous
"""Trainium2 Bass kernel for AdapterFunnelRelMultiheadAttention (v3).

Sharding: data-parallel over (batch, query-block). 8 cores; core c handles
batch c//4, query rows [(c%4)*256, (c%4)*256+256), all 12 heads. No
collectives; host slices inputs (contraction-major, pre-tiled to the SBUF
partition layout) and concatenates the 8 outputs.

v3 changes over v2 (205us baseline):
  - q/k/v/r_head projections run as fp8e4 DoubleRow matmuls (contraction
    256 per pass, half the PE streaming cycles); weights are host-scaled
    x16 into the fp8 dynamic range and descaled in the PSUM drains. All
    fp8 inputs also halve the HBM load volume.
  - content matmuls contract over 64 partitions only, so the e0/e1 head
    halves run as concurrent row-tiles (disjoint PE row groups, separate
    PSUM banks); pos matmuls are row-tiled the same way.
  - the token-type t2 = ttm*sd + band elementwise op moves from the
    (saturated) vector engine to the otherwise-idle gpsimd engine.
  - content+inject PSUM is one [128,1024] 2-bank tile per head half, so
    the exp reads it in a single activation with one accumulator read.
  - PSUM->SBUF drains are split between vector and scalar engines.
"""

import math

import numpy as np

B, S, D, H, DH, A = 2, 1024, 768, 12, 64, 64
C = S
N_CORES = 8
IB = (B * S) // N_CORES        # 256 query rows per core
NT_I = IB // 128               # 2 i-tiles
NT_J = C // 128                # 8 j-tiles
NT_D = D // 128                # 6 contraction tiles
NT_H = (H * DH) // 128         # 6 head-pair tiles
TW = 1280                      # padded rel-pos window rows per core
PW = 1152                      # per-i-tile pos window width (512+512+128)
NCONST = 6 * NT_H + NT_I       # packed constants: bq bk bpost rwb rrb rsb rmask
SCALE = 1.0 / (DH ** 0.5)
WS = 16.0                      # host-side fp8 weight scale
RW = 1.0 / WS

_CACHE = {}


def _build_graph():
    from contextlib import ExitStack

    import concourse.bass as bass
    import concourse.mybir as mybir
    import concourse.tile as tile
    from concourse import bacc

    f32 = mybir.dt.float32
    bf16 = mybir.dt.bfloat16
    fp8 = mybir.dt.float8e4
    op = mybir.AluOpType
    AF = mybir.ActivationFunctionType
    DR = mybir.MatmulPerfMode.DoubleRow
    ds = bass.ds

    nc = bacc.Bacc()

    # ---- per-core DRAM parameters (pre-tiled [128, nt, w] layouts) -----
    qr_p = nc.declare_dram_parameter("q_rows", [128, NT_I, D], f32, isOutput=False)
    qrt_p = nc.declare_dram_parameter("q_rowsT", [128, NT_D, IB], fp8, isOutput=False)
    keyt_p = nc.declare_dram_parameter("keyT", [128, NT_D, C], fp8, isOutput=False)
    valt_p = nc.declare_dram_parameter("valT", [128, NT_D, C], fp8, isOutput=False)
    post_p = nc.declare_dram_parameter("posT", [128, NT_D, TW], fp8, isOutput=False)
    ttm_p = nc.declare_dram_parameter("ttm", [128, NT_I, C], bf16, isOutput=False)
    wqt_p = nc.declare_dram_parameter("wqT", [128, NT_D, H * DH], fp8, isOutput=False)
    wkt_p = nc.declare_dram_parameter("wkT", [128, NT_D, H * DH], fp8, isOutput=False)
    wvt_p = nc.declare_dram_parameter("wvT", [128, NT_D, H * DH], fp8, isOutput=False)
    rk_p = nc.declare_dram_parameter("rk", [128, NT_D, H * DH], fp8, isOutput=False)
    wpt_p = nc.declare_dram_parameter("wpostT", [128, NT_H, D], bf16, isOutput=False)
    wdt_p = nc.declare_dram_parameter("wdownT", [128, NT_D, A], bf16, isOutput=False)
    wut_p = nc.declare_dram_parameter("wupT", [A, D], bf16, isOutput=False)
    segb_p = nc.declare_dram_parameter("seg_b", [128, NT_H, 4], bf16, isOutput=False)
    idm_p = nc.declare_dram_parameter("idm", [128, 128], bf16, isOutput=False)
    const_p = nc.declare_dram_parameter("consts", [128, NCONST], f32, isOutput=False)
    lnx_p = nc.declare_dram_parameter("lnx", [1, 3, D], f32, isOutput=False)
    out_p = nc.declare_dram_parameter("out", [IB, D], f32, isOutput=True)

    out_d = out_p.ap()

    with tile.TileContext(nc) as tc, ExitStack() as ctx:
        per = ctx.enter_context(tc.tile_pool(name="persist", bufs=1))

        # ---- persistent tiles -----------------------------------------
        query_nat = per.tile([128, NT_I, D], f32, tag="qnat", name="qnat")
        qTw = per.tile([128, NT_H, IB], bf16, tag="qTw", name="qTw")
        qTr = per.tile([128, NT_H, IB], bf16, tag="qTr", name="qTr")
        kT = per.tile([128, NT_H, C], bf16, tag="kT", name="kT")
        v_sb = per.tile([128, NT_J, H * DH], bf16, tag="vsb", name="vsb")
        rhT = per.tile([128, NT_H, TW], bf16, tag="rhT", name="rhT")
        ttm_bf = per.tile([128, NT_I, C], bf16, tag="ttm", name="ttm")
        segB = per.tile([128, NT_H, 4], bf16, tag="segB", name="segB")
        id_bf = per.tile([128, 128], bf16, tag="id_bf", name="id_bf")
        avT = per.tile([128, NT_H, IB], bf16, tag="avT", name="avT")
        aoT = per.tile([128, NT_H, IB], bf16, tag="aoT", name="aoT")
        a2T = per.tile([128, NT_H, IB], bf16, tag="a2T", name="a2T")
        wpT = per.tile([128, NT_H, D], bf16, tag="wpT", name="wpT")
        wdT = per.tile([128, NT_D, A], bf16, tag="wdT", name="wdT")
        wuT = per.tile([64, D], bf16, tag="wuT", name="wuT")
        consts = per.tile([128, NCONST], f32, tag="consts", name="consts")
        ttsc = per.tile([128, NT_I * NT_H * 4], f32, tag="ttsc", name="ttsc")
        sd_all = per.tile([128, NT_I * NT_H * 2], f32, tag="sdall", name="sdall")
        dd_all = per.tile([128, NT_I * NT_H * 2], f32, tag="ddall", name="ddall")
        qb_w = per.tile([128, NT_H], f32, tag="qb_w", name="qb_w")
        qb_r = per.tile([128, NT_H], f32, tag="qb_r", name="qb_r")
        qb_s = per.tile([128, NT_H], f32, tag="qb_s", name="qb_s")
        lnx_row = per.tile([1, 3, D], f32, tag="lnxr", name="lnxr")
        lnx_bc = per.tile([128, 3, D], f32, tag="lnx_bc", name="lnx_bc")
        eps_t = per.tile([128, 1], f32, tag="eps", name="eps")

        # packed constant views
        bq_t = consts[:, 0 * NT_H:1 * NT_H]
        bk_t = consts[:, 1 * NT_H:2 * NT_H]
        bpost_t = consts[:, 2 * NT_H:3 * NT_H]
        rwb_t = consts[:, 3 * NT_H:4 * NT_H]
        rrb_t = consts[:, 4 * NT_H:5 * NT_H]
        rsb_t = consts[:, 5 * NT_H:6 * NT_H]
        rmask_sb = consts[:, 6 * NT_H:6 * NT_H + NT_I]
        bv_bc = lnx_bc[:, 0, :]
        lnw_bc = lnx_bc[:, 1, :]
        lnb_bc = lnx_bc[:, 2, :]

        # =============== stage 0/1: loads + projections =================
        with (
            tc.tile_pool(name="ps1", bufs=2, space="PSUM") as ps1,
            tc.tile_pool(name="w1", bufs=1) as w1,
        ):
            queryT = w1.tile([128, NT_D, IB], fp8, tag="queryT", name="queryT")
            wqT = w1.tile([128, NT_D, H * DH], fp8, tag="wqT", name="wqT")
            keyT = w1.tile([128, NT_D, C], fp8, tag="keyT", name="keyT")
            wkT = w1.tile([128, NT_D, H * DH], fp8, tag="wkT", name="wkT")
            valT = w1.tile([128, NT_D, C], fp8, tag="valT", name="valT")
            wvT = w1.tile([128, NT_D, H * DH], fp8, tag="wvT", name="wvT")
            posT = w1.tile([128, NT_D, TW], fp8, tag="posT", name="posT")
            rkb = w1.tile([128, NT_D, H * DH], fp8, tag="rkb", name="rkb")
            qTs = w1.tile([128, NT_H, IB], bf16, tag="qTs", name="qTs")

            # all loads in strict need-order, one DMA each
            nc.sync.dma_start(out=wqT, in_=wqt_p.ap())
            nc.sync.dma_start(out=queryT, in_=qrt_p.ap())
            nc.sync.dma_start(out=consts, in_=const_p.ap())
            nc.sync.dma_start(out=segB, in_=segb_p.ap())
            nc.sync.dma_start(out=wkT, in_=wkt_p.ap())
            nc.sync.dma_start(out=keyT, in_=keyt_p.ap())
            nc.sync.dma_start(out=rkb, in_=rk_p.ap())
            nc.sync.dma_start(out=posT, in_=post_p.ap())
            nc.sync.dma_start(out=wvT, in_=wvt_p.ap())
            nc.sync.dma_start(out=valT, in_=valt_p.ap())
            nc.sync.dma_start(out=ttm_bf, in_=ttm_p.ap())
            nc.sync.dma_start(out=id_bf, in_=idm_p.ap())
            nc.sync.dma_start(out=wpT, in_=wpt_p.ap())
            nc.sync.dma_start(out=wdT, in_=wdt_p.ap())
            nc.sync.dma_start(out=wuT, in_=wut_p.ap())
            nc.sync.dma_start(out=lnx_row, in_=lnx_p.ap())
            nc.gpsimd.partition_broadcast(lnx_bc, lnx_row)
            nc.sync.dma_start(out=query_nat, in_=qr_p.ap())

            nc.vector.memset(eps_t, 1e-9)
            # preload activation tables off the critical path
            warm = w1.tile([1, 1], f32, tag="warm", name="warm", bufs=1)
            nc.scalar.activation(out=warm, in_=eps_t[0:1, 0:1], func=AF.Gelu)
            nc.scalar.activation(out=warm, in_=eps_t[0:1, 0:1], func=AF.Sqrt)
            nc.scalar.activation(out=warm, in_=eps_t[0:1, 0:1], func=AF.Exp)
            for qb, rb in ((qb_w, rwb_t), (qb_r, rrb_t), (qb_s, rsb_t)):
                nc.vector.tensor_tensor(qb, bq_t, rb, op.add)
                nc.vector.tensor_scalar(qb, qb, SCALE, None, op.mult)

            # q^T variants: (hd, i). DoubleRow fp8, interleaving ht pairs so
            # consecutive accumulating matmuls hit different PSUM banks
            for hp2 in range(NT_H // 2):
                qps = [ps1.tile([128, IB], f32, tag="projq", name=f"projq{e}", bufs=3)
                       for e in range(2)]
                for t in range(NT_D // 2):
                    for e in range(2):
                        ht = 2 * hp2 + e
                        nc.tensor.matmul(qps[e],
                                         wqT[:, ds(2 * t, 2), ds(ht * 128, 128)],
                                         queryT[:, ds(2 * t, 2), :],
                                         start=(t == 0), stop=(t == NT_D // 2 - 1),
                                         perf_mode=DR)
                for e in range(2):
                    ht = 2 * hp2 + e
                    nc.vector.tensor_scalar(qTw[:, ht, :], qps[e], SCALE * RW, qb_w[:, ht:ht + 1], op.mult, op.add)
                    nc.vector.tensor_scalar(qTr[:, ht, :], qps[e], SCALE * RW, qb_r[:, ht:ht + 1], op.mult, op.add)
                    nc.vector.tensor_scalar(qTs[:, ht, :], qps[e], SCALE * RW, qb_s[:, ht:ht + 1], op.mult, op.add)

            # token-type bias scalars for all (it, head-pair):
            # cols [4g+2e+0, 4g+2e+1] = (diff, same) for unit (g, e)
            tt_ps = ps1.tile([128, NT_I * NT_H * 4], f32, tag="tt2", name="tt2", bufs=1)
            for it in range(NT_I):
                for ht in range(NT_H):
                    g = it * NT_H + ht
                    nc.tensor.matmul(tt_ps[:, ds(g * 4, 4)],
                                     qTs[:, ht, ds(it * 128, 128)],
                                     segB[:, ht, :], start=True, stop=True)
            nc.vector.tensor_copy(out=ttsc, in_=tt_ps)
            tt4 = ttsc.rearrange("p (g e t) -> p g e t", e=2, t=2)
            nc.vector.tensor_tensor(sd_all.rearrange("p (g e) -> p g e", e=2),
                                    tt4[:, :, :, 1], tt4[:, :, :, 0], op.subtract)
            # dd = diff * rowmask (per-unit exp bias)
            for it in range(NT_I):
                usl = ds(it * NT_H * 2, NT_H * 2)
                nc.vector.tensor_scalar(
                    dd_all[:, usl].rearrange("p (g e) -> p g e", e=2),
                    tt4[:, it * NT_H:(it + 1) * NT_H, :, 0],
                    rmask_sb[:, it:it + 1], None, op.mult)

            # k^T: (hd, j), DoubleRow, bank-alternating within the 2-bank psum
            for ht in range(NT_H):
                k_ps = ps1.tile([128, C], f32, tag="proj", name="projk")
                for t in range(NT_D // 2):
                    for nh in range(2):
                        nc.tensor.matmul(k_ps[:, ds(nh * 512, 512)],
                                         wkT[:, ds(2 * t, 2), ds(ht * 128, 128)],
                                         keyT[:, ds(2 * t, 2), ds(nh * 512, 512)],
                                         start=(t == 0), stop=(t == NT_D // 2 - 1),
                                         perf_mode=DR)
                nc.vector.tensor_scalar(kT[:, ht, :], k_ps, RW, bk_t[:, ht:ht + 1], op.mult, op.add)

            # r_head^T: (hd, t), DoubleRow
            for ht in range(NT_H):
                for th in range(2):
                    r_full = ps1.tile([128, C], f32, tag="proj", name="rproj")
                    r_ps = r_full[:, 0:TW // 2]
                    for t in range(NT_D // 2):
                        for (o, w) in ((0, 512), (512, 128)):
                            nc.tensor.matmul(r_ps[:, ds(o, w)],
                                             rkb[:, ds(2 * t, 2), ds(ht * 128, 128)],
                                             posT[:, ds(2 * t, 2), ds(th * 640 + o, w)],
                                             start=(t == 0), stop=(t == NT_D // 2 - 1),
                                             perf_mode=DR)
                    nc.scalar.activation(out=rhT[:, ht, ds(th * 640, 640)], in_=r_ps,
                                         func=AF.Copy, scale=RW)

            # v natural: (j, hd) with fused bias+descale; head-pair ht occupies
            # cols [128ht, 128ht+128) = [head 2ht | head 2ht+1]
            for jt in range(NT_J):
                v_ps = ps1.tile([128, H * DH], f32, tag="proj", name="projv")
                for t in range(NT_D // 2):
                    for (o, w) in ((0, 512), (512, 256)):
                        nc.tensor.matmul(v_ps[:, ds(o, w)],
                                         valT[:, ds(2 * t, 2), ds(jt * 128, 128)],
                                         wvT[:, ds(2 * t, 2), ds(o, w)],
                                         start=(t == 0), stop=(t == NT_D // 2 - 1),
                                         perf_mode=DR)
                nc.vector.scalar_tensor_tensor(v_sb[:, jt, :], v_ps, RW, bv_bc,
                                               op.mult, op.add)

        # =============== stage 2 + interleaved stage 3 ==================
        # pair p = (it, ht) covers both head halves e0/e1 (row-tiled).
        # software pipeline A(p) / B(p-1) / C(p-2); stage-3 per-i-tile chains
        # reuse the stage-2 PSUM pools; the it=0 chain is issued as side-work
        # during the it=1 attention pairs.
        pairs = [(it, ht) for it in range(NT_I) for ht in range(NT_H)]

        with (
            tc.tile_pool(name="ps2c", bufs=2, space="PSUM") as ps2c,
            tc.tile_pool(name="ps2p", bufs=2, space="PSUM") as ps2p,
            tc.tile_pool(name="ps2t", bufs=1, space="PSUM") as ps2t,
            tc.tile_pool(name="ps2a", bufs=1, space="PSUM") as ps2a,
            tc.tile_pool(name="w2", bufs=3) as w2,
            tc.tile_pool(name="w3", bufs=2) as w3,
        ):
            state = {}

            def stage_a(p):
                it, ht = pairs[p]
                isl = ds(it * 128, 128)
                o0 = 128 * (1 - it)
                # pos scores over the 1152-wide window; e0/e1 run as
                # concurrent PE row-tiles into separate PSUM banks
                pos_sb = [w2.tile([128, PW], bf16, tag=f"possb{e}", name=f"possb{p}_{e}")
                          for e in range(2)]
                for ci, (o, w) in enumerate(((0, 512), (512, 512), (1024, PW - 1024))):
                    pps = [ps2p.tile([128, 512], f32, tag="pps", name=f"pps{p}_{ci}_{e}")
                           for e in range(2)]
                    for e in range(2):
                        hp = e * DH
                        nc.tensor.matmul(pps[e][:, 0:w],
                                         qTr[hp:hp + DH, ht, isl],
                                         rhT[hp:hp + DH, ht, ds(o0 + o, w)],
                                         start=True, stop=True)
                    # psum -> sbuf (bf16) with cls row-mask fused; e0 on DVE,
                    # e1 on ACT so the drains run in parallel
                    nc.vector.tensor_scalar(pos_sb[0][:, ds(o, w)], pps[0][:, 0:w],
                                            rmask_sb[:, it:it + 1], None, op.mult)
                    nc.scalar.activation(out=pos_sb[1][:, ds(o, w)], in_=pps[1][:, 0:w],
                                         func=AF.Copy, scale=rmask_sb[:, it:it + 1])
                # diagonal band extraction: band[p, j] = pos_sb[p, 127 - p + j]
                for e in range(2):
                    u = (it * NT_H + ht) * 2 + e
                    band = w2.tile([128, C], bf16, tag=f"band{e}", name=f"band{p}_{e}")
                    diag = bass.AP(tensor=pos_sb[e].tensor,
                                   offset=pos_sb[e].offset + 127,
                                   ap=[[PW - 1, 128], [1, C]])
                    nc.sync.dma_start(out=band, in_=diag)
                    # cls col-0: band[:,0] = -dd so exp's +dd bias cancels there
                    nc.vector.tensor_scalar(band[:, 0:1], dd_all[:, u:u + 1], -1.0,
                                            None, op.mult)
                    state[(p, e)] = band

            def stage_b(p):
                it, ht = pairs[p]
                isl = ds(it * 128, 128)
                if True:
                    state[("av", it, ht)] = ps2a.tile(
                        [128, 128], f32, tag="av", name=f"av{it}_{ht}")
                cps = []
                t2s = []
                for e in range(2):
                    u = (it * NT_H + ht) * 2 + e
                    band = state.pop((p, e))
                    # t2 = ttm*cls*(same-diff) + band, split into a 4x-mode
                    # tensor_scalar and a 2x-mode tensor_tensor (the fused
                    # scalar_tensor_tensor only runs in 1x mode, and the
                    # Pool engine rejects it entirely)
                    m1 = w2.tile([128, C], bf16, tag=f"m1{e}", name=f"m1{p}_{e}")
                    nc.vector.tensor_scalar(m1, ttm_bf[:, it, :],
                                            sd_all[:, u:u + 1], None, op.mult)
                    t2 = w2.tile([128, C], bf16, tag=f"t2{e}", name=f"t2{p}_{e}")
                    nc.vector.tensor_tensor(t2, m1, band, op.add)
                    t2s.append(t2)
                    cps.append(ps2c.tile([128, C], f32, tag="cps", name=f"cps{p}_{e}"))
                # content: e0/e1 concurrent row-tiles; halves alternate banks
                for hf in range(2):
                    for e in range(2):
                        hp = e * DH
                        nc.tensor.matmul(cps[e][:, ds(hf * 512, 512)],
                                         qTw[hp:hp + DH, ht, isl],
                                         kT[hp:hp + DH, ht, ds(hf * 512, 512)],
                                         start=True, stop=False)
                # band+tt injection on the PE (K=128), bank-alternating
                for hf in range(2):
                    for e in range(2):
                        nc.tensor.matmul(cps[e][:, ds(hf * 512, 512)], id_bf,
                                         t2s[e][:, ds(hf * 512, 512)],
                                         start=False, stop=True)
                # exp straight from the 2-bank PSUM with the token-type diff
                # bias; free softmax denominator via accum_out
                for e in range(2):
                    u = (it * NT_H + ht) * 2 + e
                    prob = w2.tile([128, C], bf16, tag=f"prob{e}", name=f"prob{p}_{e}")
                    dens = w2.tile([128, 1], f32, tag=f"dens{e}", name=f"dens{p}_{e}")
                    nc.scalar.activation(out=prob, in_=cps[e],
                                         func=AF.Exp, bias=dd_all[:, u:u + 1],
                                         accum_out=dens)
                    rd = w2.tile([128, 1], f32, tag=f"rd{e}", name=f"rd{p}_{e}")
                    nc.vector.reciprocal(rd, dens)
                    prob_n = w2.tile([128, C], bf16, tag=f"probn{e}", name=f"probn{p}_{e}")
                    nc.vector.tensor_scalar(prob_n, prob, rd[:, 0:1], None, op.mult)
                    state[(p, e)] = prob_n

            def stage_c(p):
                it, ht = pairs[p]
                av = state[("av", it, ht)]
                for e in range(2):
                    prob_n = state.pop((p, e))
                    hp = e * DH
                    pT_ps = ps2t.tile([128, NT_J, 128], bf16, tag="pT", name=f"pT{p}_{e}")
                    for jc in range(NT_J):
                        nc.tensor.transpose(pT_ps[:, jc, :], prob_n[:, ds(jc * 128, 128)],
                                            id_bf)
                    probT = w2.tile([128, NT_J, 128], bf16, tag=f"probT{e}",
                                    name=f"probT{p}_{e}")
                    if e == 0:
                        nc.vector.tensor_copy(out=probT, in_=pT_ps)
                    else:
                        nc.scalar.copy(out=probT, in_=pT_ps)
                    # PV accumulation; head e lands on psum partitions [64e, 64e+64)
                    for jt in range(NT_J):
                        nc.tensor.matmul(av[hp:hp + DH, :],
                                         v_sb[:, jt, ds(ht * 128 + hp, DH)],
                                         probT[:, jt, :],
                                         start=(jt == 0), stop=(jt == NT_J - 1))
                avp = state.pop(("av", it, ht))
                nc.vector.tensor_copy(out=avT[:, ht, ds(it * 128, 128)], in_=avp)

            # ---- stage-3 side-work chain for one i-tile ----------------
            def stage3_work(it):
                isl = ds(it * 128, 128)
                # post-projection per output tile. PSUM comes from the "pps"
                # tag: its buffer frees are vector ops issued earlier in the
                # queue, so side-work never waits on future PE work.
                for et in range(NT_H):
                    po_ps = ps2p.tile([128, 512], f32, tag="pps", name=f"po{it}_{et}")
                    for kc in range(NT_H):
                        nc.tensor.matmul(po_ps[:, 0:128], wpT[:, kc, ds(et * 128, 128)],
                                         avT[:, kc, isl],
                                         start=(kc == 0), stop=(kc == NT_H - 1))
                    nc.vector.tensor_scalar(aoT[:, et, isl], po_ps[:, 0:128],
                                            bpost_t[:, et:et + 1], None, op.add)
                    yield
                # adapter down -> gelu -> up -> residual
                z_full = ps2p.tile([128, 512], f32, tag="pps", name=f"z{it}")
                z_ps = z_full[:, 0:128]
                for kc in range(NT_D):
                    nc.tensor.matmul(z_ps[0:DH, :], wdT[:, kc, 0:A], aoT[:, kc, isl],
                                     start=(kc == 0), stop=(kc == NT_D - 1))
                gT = w3.tile([DH, 128], bf16, tag="gT", name=f"gT{it}")
                nc.scalar.activation(out=gT, in_=z_ps[0:DH, :], func=AF.Gelu)
                yield
                for et in range(NT_H):
                    u_ps = ps2p.tile([128, 512], f32, tag="pps", name=f"up{it}_{et}")
                    nc.tensor.matmul(u_ps[:, 0:128], wuT[0:A, ds(et * 128, 128)], gT,
                                     start=True, stop=True)
                    nc.vector.tensor_tensor(a2T[:, et, isl], u_ps[:, 0:128],
                                            aoT[:, et, isl], op.add)
                    yield
                # transpose back to natural layout on the PE
                n_ps = ps2t.tile([128, NT_J, 128], bf16, tag="pT", name=f"trn{it}")
                for et in range(NT_H):
                    nc.tensor.transpose(n_ps[:, et, :], a2T[:, et, isl], id_bf)
                a2n = w3.tile([128, D], bf16, tag="a2n", name=f"a2n{it}")
                nc.vector.tensor_copy(out=a2n, in_=n_ps[:, 0:NT_H, :])
                yield
                x = w3.tile([128, D], f32, tag="x", name=f"x{it}")
                nc.vector.tensor_tensor(x, query_nat[:, it, :], a2n, op.add)
                stats = w3.tile([128, 3, 6], f32, tag="stats", name=f"stats{it}")
                for c3 in range(3):
                    nc.vector.bn_stats(stats[:, c3, :], x[:, ds(c3 * 256, 256)])
                mv = w3.tile([128, 2], f32, tag="mv", name=f"mv{it}")
                nc.vector.bn_aggr(mv, stats)
                sstd = w3.tile([128, 1], f32, tag="sstd", name=f"sstd{it}")
                nc.scalar.activation(out=sstd, in_=mv[:, 1:2], func=AF.Sqrt,
                                     bias=eps_t[:, 0:1], scale=1.0)
                rstd = w3.tile([128, 1], f32, tag="rstd", name=f"rstd{it}")
                nc.vector.reciprocal(rstd, sstd)
                xa = w3.tile([128, D], f32, tag="xa", name=f"xa{it}")
                nc.vector.scalar_tensor_tensor(xa, x, mv[:, 0:1], lnw_bc,
                                               op.subtract, op.mult)
                ot = w3.tile([128, D], f32, tag="ot", name=f"ot{it}")
                nc.vector.scalar_tensor_tensor(ot, xa, rstd, lnb_bc, op.mult, op.add)
                nc.sync.dma_start(out=out_d[ds(it * 128, 128), :], in_=ot)
                yield

            side = None
            np_ = len(pairs)
            for p in range(np_):
                stage_a(p)
                if p >= 1:
                    stage_b(p - 1)
                if p >= 2:
                    stage_c(p - 2)
                    if pairs[p - 2] == (0, NT_H - 1):
                        side = stage3_work(0)   # it=0 attention fully done
                    if side is not None:
                        next(side, None)
                        next(side, None)
            stage_b(np_ - 1)
            for p in range(np_ - 2, np_):
                stage_c(p)
                if side is not None:
                    next(side, None)
                    next(side, None)
            if side is not None:
                for _ in side:
                    pass
            for _ in stage3_work(1):
                pass

    nc.compile()
    return nc


def _make_seg_b(seg):
    """Block-diagonal seg operand: one (128, 4) tile per head pair so the
    token-type bias matmul contracts over the full 128 partitions."""
    sb = np.zeros((128, NT_H, 4), np.float32)
    for ht in range(NT_H):
        h0, h1 = 2 * ht, 2 * ht + 1
        sb[0:DH, ht, 0] = seg[0, h0 * DH:(h0 + 1) * DH]
        sb[0:DH, ht, 1] = seg[1, h0 * DH:(h0 + 1) * DH]
        sb[DH:128, ht, 2] = seg[0, h1 * DH:(h1 + 1) * DH]
        sb[DH:128, ht, 3] = seg[1, h1 * DH:(h1 + 1) * DH]
    return sb


def _reference_numpy(inputs):
    """Exact numpy fallback (never hit for the spec'd generator; guards the
    amask==1 / factorizable-cls assumptions)."""
    f = np.float64
    q = np.asarray(inputs["query"], f)
    k = np.asarray(inputs["key"], f)
    v = np.asarray(inputs["value"], f)
    pe = np.asarray(inputs["pos_embed"], f)
    ttm = np.asarray(inputs["token_type_mat"]).astype(bool)
    am = np.asarray(inputs["attention_mask"], f)
    cls = np.asarray(inputs["cls_mask"], f)
    wq, bq = np.asarray(inputs["wq"], f), np.asarray(inputs["bq"], f)
    wk, bk = np.asarray(inputs["wk"], f), np.asarray(inputs["bk"], f)
    wv, bv = np.asarray(inputs["wv"], f), np.asarray(inputs["bv"], f)
    rwb, rrb = np.asarray(inputs["r_w_bias"], f), np.asarray(inputs["r_r_bias"], f)
    rsb = np.asarray(inputs["r_s_bias"], f)
    rk = np.asarray(inputs["r_kernel"], f)
    seg = np.asarray(inputs["seg_embed"], f)
    wpo, bpo = np.asarray(inputs["w_post"], f), np.asarray(inputs["b_post"], f)
    wd, wu = np.asarray(inputs["w_down"], f), np.asarray(inputs["w_up"], f)
    lw, lb = np.asarray(inputs["ln_w"], f), np.asarray(inputs["ln_b"], f)
    b, s, d = q.shape
    c = k.shape[1]
    h, dh = rwb.shape
    scale = 1.0 / dh ** 0.5
    qp = (q @ wq.T + bq).reshape(b, s, h, dh) * scale
    kp = (k @ wk.T + bk).reshape(b, c, h, dh)
    vp = (v @ wv.T + bv).reshape(b, c, h, dh)
    content = np.einsum('bind,bjnd->bnij', qp + rwb * scale, kp)
    r_head = np.einsum('td,dnh->tnh', pe, rk)
    pos = np.einsum('binh,tnh->bnit', qp + rrb * scale, r_head)
    t = pos.shape[-1]
    pos = pos.reshape(b, h, t, s)[:, :, 1:, :].reshape(b, h, s, t - 1)[..., :c]
    pos = pos * cls
    ttb = np.einsum('bind,snd->bnis', qp + rsb * scale, seg)
    tta = np.where(ttm[:, None], ttb[..., 1:2], ttb[..., 0:1]) * cls
    sc = content + pos + tta - 1e6 * (1.0 - am[:, None, None])
    sc = sc - sc.max(-1, keepdims=True)
    p = np.exp(sc)
    p /= p.sum(-1, keepdims=True)
    av = np.einsum('bnij,bjnd->bind', p, vp).reshape(b, s, h * dh)
    ao = av @ wpo.T + bpo
    z = ao @ wd.T
    erf = np.vectorize(math.erf)
    g = z * 0.5 * (1.0 + erf(z / np.sqrt(2.0)))
    ao = g @ wu.T + ao
    x = q + ao
    mu = x.mean(-1, keepdims=True)
    var = x.var(-1, keepdims=True)
    return ((x - mu) / np.sqrt(var + 1e-9) * lw + lb).astype(np.float32)


def _pack_consts(inputs, rowmask, i0):
    """[128, NCONST] f32: bq bk bpost rwb rrb rsb (6 x NT_H cols) + rowmask."""
    f = np.float32
    cst = np.zeros((128, NCONST), f)
    for ci, key in enumerate(("bq", "bk", "b_post")):
        cst[:, ci * NT_H:(ci + 1) * NT_H] = (
            np.asarray(inputs[key], f).reshape(NT_H, 128).T)
    for ci, key in enumerate(("r_w_bias", "r_r_bias", "r_s_bias")):
        cst[:, (3 + ci) * NT_H:(4 + ci) * NT_H] = (
            np.asarray(inputs[key], f).reshape(NT_H, 128).T)
    cst[:, 6 * NT_H:] = rowmask[i0:i0 + IB].reshape(NT_I, 128).T
    return cst


def _tile_p(x, w):
    """[128*nt, w] -> [128, nt, w] partition-tile layout."""
    nt = x.shape[0] // 128
    return np.ascontiguousarray(x.reshape(nt, 128, w).transpose(1, 0, 2))


def _shard_inputs(inputs):
    """Slice full inputs into 8 per-core input maps (contraction-major,
    pre-tiled and pre-cast to fp8/bf16 host-side)."""
    import ml_dtypes
    f = np.float32
    bf = ml_dtypes.bfloat16
    f8 = ml_dtypes.float8_e4m3fn

    def t8(x, w, scale=1.0):  # fp8 + partition-tiled
        return _tile_p((np.asarray(x, f) * scale).astype(f8), w)

    q = np.asarray(inputs["query"], dtype=f)
    k = np.asarray(inputs["key"], dtype=f)
    v = np.asarray(inputs["value"], dtype=f)
    pe = np.asarray(inputs["pos_embed"], dtype=f)
    ttm = np.asarray(inputs["token_type_mat"]).astype(f)
    cls = np.asarray(inputs["cls_mask"], dtype=f)

    # factor cls = outer(rowmask, colmask); verified by _check_fastpath
    jref = int(np.argmax(cls[S // 2]))
    rowmask = cls[:, jref].copy()
    lnx = np.stack([np.asarray(inputs["bv"], f), np.asarray(inputs["ln_w"], f),
                    np.asarray(inputs["ln_b"], f)])[None]
    shared = {
        "wqT": t8(np.asarray(inputs["wq"], f).T, H * DH, WS),
        "wkT": t8(np.asarray(inputs["wk"], f).T, H * DH, WS),
        "wvT": t8(np.asarray(inputs["wv"], f).T, H * DH, WS),
        "rk": t8(np.asarray(inputs["r_kernel"], f).reshape(D, H * DH), H * DH, WS),
        "wpostT": _tile_p(np.asarray(inputs["w_post"], f).T.astype(bf), D),
        "wdownT": _tile_p(np.asarray(inputs["w_down"], f).T.astype(bf), A),
        "wupT": np.asarray(inputs["w_up"], f).T.astype(bf),
        "seg_b": _make_seg_b(np.asarray(inputs["seg_embed"], f).reshape(2, H * DH)).astype(bf),
        "idm": np.eye(128, dtype=f).astype(bf),
        "lnx": lnx,
    }
    in_maps = []
    for cidx in range(N_CORES):
        b, i0 = cidx // (N_CORES // B), (cidx % (N_CORES // B)) * IB
        win = pe[769 - i0: 2048 - i0]
        if win.shape[0] < TW:
            win = np.concatenate([win, np.zeros((TW - win.shape[0], D), f)], axis=0)
        m = dict(shared)
        m["q_rows"] = _tile_p(q[b, i0:i0 + IB], D)
        m["q_rowsT"] = t8(q[b, i0:i0 + IB].T, IB)
        m["keyT"] = t8(k[b].T, C)
        m["valT"] = t8(v[b].T, C)
        m["posT"] = t8(win.T, TW)
        m["ttm"] = _tile_p((ttm[b, i0:i0 + IB] * cls[i0:i0 + IB]).astype(bf), C)
        m["consts"] = _pack_consts(inputs, rowmask, i0)
        in_maps.append(m)
    return in_maps


def _check_fastpath(inputs):
    cls = np.asarray(inputs["cls_mask"], dtype=np.float32)
    am = np.asarray(inputs["attention_mask"], dtype=np.float32)
    if not np.all(am == 1.0):
        return False
    iref = int(np.argmax(cls[:, S // 2]))
    jref = int(np.argmax(cls[S // 2]))
    rowmask = cls[:, jref]
    colmask = cls[iref, :]
    if not np.array_equal(cls, np.outer(rowmask, colmask)):
        return False
    # graph hardcodes: col mask zero exactly at j=0, one elsewhere
    if not (colmask[0] == 0.0 and np.all(colmask[1:] == 1.0)):
        return False
    return True


def _run(inputs, trace=False):
    from concourse.bass_utils import run_bass_kernel_spmd

    if "nc" not in _CACHE:
        _CACHE["nc"] = _build_graph()
    nc = _CACHE["nc"]
    in_maps = _shard_inputs(inputs)
    res = run_bass_kernel_spmd(nc, in_maps, core_ids=list(range(N_CORES)), trace=trace)
    out = np.empty((B, S, D), np.float32)
    for c in range(N_CORES):
        b, i0 = c // (N_CORES // B), (c % (N_CORES // B)) * IB
        out[b, i0:i0 + IB] = res.results[c]["out"]
    return out, res


def kernel(**inputs):
    if not _check_fastpath(inputs):
        return _reference_numpy(inputs)
    out, _ = _run(inputs, trace=False)
    return out
